# revision 28
# baseline (speedup 1.0000x reference)
"""Multi-head self-attention (B=4, S=2048, D=1024, H=16) on 8 TRN2 cores.

Sharding: core c handles batch b=c//2, query rows [h*1024, (h+1)*1024) with
h=c%2, for ALL 16 heads. K/V are computed per-core over the full sequence
(duplicated across the 2 cores of a batch), so there are no collectives and
the host-side unshard is a pure concatenation.

Host prep per core: xT = x[b].T rolled so the core's queries occupy columns
0..1023 (attention is invariant to key/value ordering, so K/V over the
rolled sequence give identical outputs).

Device layout choices (everything contracts on the partition dim):
  - Q^T, K^T computed as [dq, s] / [dk, s] via lhsT=W (natural), rhs=xT.
  - scores^T [sk, sq] via lhsT=K^T tile (K=64 contraction; two heads of a
    pair live at partitions 0:64 / 64:128 and pack the PE array).
  - exp on ScalarE with fused 1/8 scale, bf16 out.
  - attn^T via lhsT=[V|1] (65 cols): row 64 accumulates the softmax
    denominators for free.
  - normalize with DVE reciprocal + GpSimd partition_broadcast + DVE mul.
  - out[sq, do] via lhsT=attn^T tile (natural output layout, contiguous DMA).
"""

import numpy as np
from contextlib import ExitStack

import concourse.bass as bass
import concourse.mybir as mybir
import concourse.tile as tile
from concourse.bass_utils import run_bass_kernel_spmd
from concourse.vector_clock import ScopedClock

F32 = mybir.dt.float32
BF16 = mybir.dt.bfloat16

D = 1024
S = 2048
SQ = 1024  # local query rows per core
H = 16
HD = 64
NPAIR = 8  # head pairs; pair p = heads (2p, 2p+1) = Q/K rows 128p..128p+127
NCORES = 8

# ---------------------------------------------------------------------------
# Workaround: this walrus build rejects >1 sem-wait per instruction ("Too
# many sync wait commands"). After the kernel is fully built, hoist excess
# waits from every instruction onto single-wait NOPs inserted just before it
# in the same engine stream (per-engine program order is preserved, so
# blocking on the NOPs first is equivalent).
# ---------------------------------------------------------------------------


def _split_all_multiwaits(nc):
    n = 0
    for fn in nc.m.functions:
        for bb in fn.blocks:
            lst = bb.instructions
            i = 0
            while i < len(lst):
                inst = lst[i]
                si = inst.sync_info
                if si is not None and si.on_wait is not None and len(si.on_wait) > 1:
                    waits = list(si.on_wait)
                    keep = waits[-1:]
                    del si.on_wait[:]
                    si.on_wait.extend(keep)
                    nops = []
                    for w in waits[:-1]:
                        nop = mybir.InstNoOp(name=f"WSPL-{n}", ins=[], outs=[])
                        n += 1
                        nop.engine = inst.engine
                        nop.sync_info = mybir.SyncInfo(on_wait=[w], on_update=[])
                        nops.append(nop)
                    lst[i:i] = nops
                    i += len(nops)
                i += 1
    return n


# ---------------------------------------------------------------------------
# Kernel build
# ---------------------------------------------------------------------------


def _bcast_ap(dram_handle, nparts, offset_elems, n):
    """DRAM AP replicating a [n] vector across nparts partitions."""
    return bass.AP(
        tensor=dram_handle,
        offset=offset_elems,
        ap=[[0, nparts], [1, n]],
    )


def build_kernel():
    nc = bass.Bass()

    xT = nc.declare_dram_parameter("xT", [D, S], BF16, isOutput=False)
    Wq = nc.declare_dram_parameter("Wq", [D, D], BF16, isOutput=False)
    Wk = nc.declare_dram_parameter("Wk", [D, D], BF16, isOutput=False)
    Wv = nc.declare_dram_parameter("Wv", [D, D], BF16, isOutput=False)
    Wo = nc.declare_dram_parameter("Wo", [D, D], BF16, isOutput=False)
    bq = nc.declare_dram_parameter("bq", [D], F32, isOutput=False)
    bk = nc.declare_dram_parameter("bk", [D], F32, isOutput=False)
    bv = nc.declare_dram_parameter("bv", [D], F32, isOutput=False)
    bo = nc.declare_dram_parameter("bo", [D], F32, isOutput=False)
    out = nc.declare_dram_parameter("out", [SQ, D], F32, isOutput=True)

    Exp = mybir.ActivationFunctionType.Exp

    with tile.TileContext(nc) as tc:
        with ExitStack() as ctx:
            const = ctx.enter_context(tc.tile_pool(name="const", bufs=1))
            xpool = ctx.enter_context(tc.tile_pool(name="xres", bufs=1))
            wqk = ctx.enter_context(tc.tile_pool(name="wqk", bufs=1))
            wv_pool = ctx.enter_context(tc.tile_pool(name="wv", bufs=2))
            qk_pool = ctx.enter_context(tc.tile_pool(name="qk", bufs=2))
            vg_pool = ctx.enter_context(tc.tile_pool(name="vg", bufs=2))
            exp_pool = ctx.enter_context(tc.tile_pool(name="expp", bufs=3))
            small = ctx.enter_context(tc.tile_pool(name="small", bufs=1))
            out_pool = ctx.enter_context(tc.tile_pool(name="outp", bufs=2))
            wo_pool = ctx.enter_context(tc.tile_pool(name="wo", bufs=3))

            spsum = ctx.enter_context(tc.tile_pool(name="sp", bufs=2, space="PSUM"))
            apsum = ctx.enter_context(tc.tile_pool(name="ap", bufs=1, space="PSUM"))
            qpsum = ctx.enter_context(tc.tile_pool(name="qp", bufs=2, space="PSUM"))
            drpool = ctx.enter_context(tc.tile_pool(name="dr", bufs=4, space="DRAM"))

            # Residents: x^T [d, s] as 8 partition-tiles; attn^T accumulator.
            xT_sb = xpool.tile([128, 8, S], BF16)
            xT_r = xT.rearrange("(k p) s -> p k s", p=128)
            for k in range(8):
                nc.sync.dma_start(out=xT_sb[:, k, :], in_=xT_r[:, k, :])

            attnT = xpool.tile([128, NPAIR, SQ], BF16)

            # Biases.
            bq_sb = const.tile([128, 8], F32)
            nc.gpsimd.dma_start(out=bq_sb[:], in_=bq.rearrange("(k p) -> p k", p=128))
            bk_sb = const.tile([128, 8], F32)
            nc.gpsimd.dma_start(out=bk_sb[:], in_=bk.rearrange("(k p) -> p k", p=128))
            bv_bc = const.tile([128, D], F32)
            nc.gpsimd.dma_start(out=bv_bc[:], in_=_bcast_ap(bv, 128, 0, D))
            bo_bc = const.tile([128, D], F32)
            nc.gpsimd.dma_start(out=bo_bc[:], in_=_bcast_ap(bo, 128, 0, D))


            def emit_vgroup(g):
                wv_g = wv_pool.tile([128, 8, 512], BF16, tag="wvg")
                nc.sync.dma_start(
                    out=wv_g[:],
                    in_=Wv[:, 512 * g : 512 * (g + 1)].rearrange(
                        "(k p) c -> p k c", p=128
                    ),
                )
                vg = vg_pool.tile([128, 16, 8, 65], BF16, tag="vg")
                # ones column (index 64) via ACT: Copy(x*0 + 1) == 1.0
                nc.scalar.activation(
                    vg[:, :, :, 64:65],
                    bv_bc[:, 0:128].rearrange("p (a b c) -> p a b c", a=16, b=8),
                    mybir.ActivationFunctionType.Copy,
                    bias=1.0,
                    scale=0.0,
                )
                for skt in range(16):
                    ps = qpsum.tile([128, 512], F32, tag="qp")
                    for k in range(8):
                        nc.tensor.matmul(
                            ps[:],
                            xT_sb[:, k, 128 * skt : 128 * (skt + 1)],
                            wv_g[:, k, :],
                            start=(k == 0),
                            stop=(k == 7),
                        )
                    nc.vector.tensor_add(
                        vg[:, skt, :, 0:64],
                        ps[:].rearrange("p (h d) -> p h d", h=8),
                        bv_bc[:, 512 * g : 512 * (g + 1)].rearrange(
                            "p (h d) -> p h d", h=8
                        ),
                    )
                return vg

            vg = emit_vgroup(0)
            vg_next = None
            for p in range(NPAIR):
                g, pl = divmod(p, 4)  # V group, pair-local index
                if p == 2:
                    vg_next = emit_vgroup(1)
                if p == 4:
                    vg = vg_next

                # --- Q^T / K^T for this pair ---
                wq_p = wqk.tile([128, 8, 128], BF16, tag="wq")
                nc.sync.dma_start(
                    out=wq_p[:],
                    in_=Wq[:, 128 * p : 128 * (p + 1)].rearrange(
                        "(k p) c -> p k c", p=128
                    ),
                )
                wk_p = wqk.tile([128, 8, 128], BF16, tag="wk")
                nc.sync.dma_start(
                    out=wk_p[:],
                    in_=Wk[:, 128 * p : 128 * (p + 1)].rearrange(
                        "(k p) c -> p k c", p=128
                    ),
                )

                qt = qk_pool.tile([128, SQ], BF16, tag="qt")
                for c in range(SQ // 512):
                    ps = qpsum.tile([128, 512], F32, tag="qp")
                    for k in range(8):
                        nc.tensor.matmul(
                            ps[:],
                            wq_p[:, k, :],
                            xT_sb[:, k, 512 * c : 512 * (c + 1)],
                            start=(k == 0),
                            stop=(k == 7),
                        )
                    nc.vector.tensor_scalar_add(
                        qt[:, 512 * c : 512 * (c + 1)], ps[:], bq_sb[:, p : p + 1]
                    )

                kt = qk_pool.tile([128, S], BF16, tag="kt")
                for c in range(S // 512):
                    ps = qpsum.tile([128, 512], F32, tag="qp")
                    for k in range(8):
                        nc.tensor.matmul(
                            ps[:],
                            wk_p[:, k, :],
                            xT_sb[:, k, 512 * c : 512 * (c + 1)],
                            start=(k == 0),
                            stop=(k == 7),
                        )
                    nc.vector.tensor_scalar_add(
                        kt[:, 512 * c : 512 * (c + 1)], ps[:], bk_sb[:, p : p + 1]
                    )

                # --- attention for the two heads of this pair ---
                sums97 = small.tile([97, 512], F32, tag="sums")
                for cq in range(SQ // 512):
                    attnA = apsum.tile([65, 512], F32, tag="aA")
                    attnB = apsum.tile([65, 512], F32, tag="aB")

                    def emit_attnv(skt, exs):
                        nc.tensor.matmul(
                            attnA[:],
                            vg[:, skt, 2 * pl, :],
                            exs[:, 0:512],
                            start=(skt == 0),
                            stop=(skt == 15),
                        )
                        nc.tensor.matmul(
                            attnB[:],
                            vg[:, skt, 2 * pl + 1, :],
                            exs[:, 512:1024],
                            start=(skt == 0),
                            stop=(skt == 15),
                        )

                    prev = None
                    for skt in range(16):
                        sc = spsum.tile([128, 1024], F32, tag="sc")
                        nc.tensor.matmul(
                            sc[:, 0:512],
                            kt[0:64, 128 * skt : 128 * (skt + 1)],
                            qt[0:64, 512 * cq : 512 * (cq + 1)],
                            start=True,
                            stop=True,
                        )
                        nc.tensor.matmul(
                            sc[:, 512:1024],
                            kt[64:128, 128 * skt : 128 * (skt + 1)],
                            qt[64:128, 512 * cq : 512 * (cq + 1)],
                            start=True,
                            stop=True,
                        )
                        ex = exp_pool.tile([128, 1024], BF16, tag="ex")
                        nc.scalar.activation(ex[:], sc[:], Exp, scale=0.125)
                        if prev is not None:
                            emit_attnv(skt - 1, prev)
                        prev = ex
                    emit_attnv(15, prev)

                    # Stash denominators (row 64) and evict attn^T
                    # unnormalized right away so the accumulation banks free
                    # quickly; normalization happens once per pair below.
                    for half, at in ((0, attnA), (1, attnB)):
                        r = 32 * (2 * cq + half)
                        nc.vector.tensor_copy(
                            sums97[r : r + 1, :], at[64:65, :]
                        )
                        nc.vector.tensor_copy(
                            attnT[
                                64 * half : 64 * (half + 1),
                                p,
                                512 * cq : 512 * (cq + 1),
                            ],
                            at[0:64, :],
                        )

                # One batched reciprocal for the pair's 4 (cq, head) chunks,
                # partition-broadcast via a DRAM round-trip (cast to bf16),
                # then normalize attn^T in place.
                rr97 = small.tile([97, 512], F32, tag="rr")
                nc.vector.reciprocal(rr97[:], sums97[:])
                scr = drpool.tile([4, 512], F32, tag="scr")
                nc.sync.dma_start(
                    out=scr[:],
                    in_=bass.AP(
                        tensor=rr97.tensor,
                        offset=rr97.offset,
                        ap=[[32 * 512, 4], [1, 512]],
                    ),
                )
                bc4 = small.tile([128, 4, 512], BF16, tag="bc")
                nc.gpsimd.dma_start(
                    out=bc4[:],
                    in_=bass.AP(
                        tensor=scr.tensor,
                        offset=scr.offset,
                        ap=[[0, 128], [512, 4], [1, 512]],
                    ),
                )
                for cq in range(SQ // 512):
                    for half in range(2):
                        sl = attnT[
                            64 * half : 64 * (half + 1),
                            p,
                            512 * cq : 512 * (cq + 1),
                        ]
                        nc.vector.tensor_mul(
                            sl,
                            sl,
                            bc4[64 * half : 64 * (half + 1), 2 * cq + half, :],
                        )



            # --- output projection: out[sq, do] += attn^T.T @ Wo ---
            OC = 512
            for c in range(D // OC):
                wo_c = wo_pool.tile([128, 8, OC], BF16, tag="wo")
                nc.sync.dma_start(
                    out=wo_c[:],
                    in_=Wo[:, OC * c : OC * (c + 1)].rearrange(
                        "(k p) c -> p k c", p=128
                    ),
                )
                for t in range(SQ // 128):
                    ps = qpsum.tile([128, OC], F32, tag="qp")
                    for k in range(8):
                        nc.tensor.matmul(
                            ps[:],
                            attnT[:, k, 128 * t : 128 * (t + 1)],
                            wo_c[:, k, :],
                            start=(k == 0),
                            stop=(k == 7),
                        )
                    ot = out_pool.tile([128, OC], F32, tag="ot")
                    nc.vector.tensor_add(
                        ot[:], ps[:], bo_bc[:, OC * c : OC * (c + 1)]
                    )
                    nc.sync.dma_start(
                        out=out[128 * t : 128 * (t + 1), OC * c : OC * (c + 1)],
                        in_=ot[:],
                    )

    _split_all_multiwaits(nc)
    return nc


_NC_CACHE = None


def _get_nc():
    global _NC_CACHE
    if _NC_CACHE is None:
        _NC_CACHE = build_kernel()
    return _NC_CACHE


def make_in_maps(x, Wq, bq, Wk, bk, Wv, bv, Wo, bo):
    import ml_dtypes

    bf16 = ml_dtypes.bfloat16
    x = np.asarray(x, dtype=np.float32)
    shared = {
        "Wq": np.ascontiguousarray(np.asarray(Wq, dtype=np.float32).astype(bf16)),
        "Wk": np.ascontiguousarray(np.asarray(Wk, dtype=np.float32).astype(bf16)),
        "Wv": np.ascontiguousarray(np.asarray(Wv, dtype=np.float32).astype(bf16)),
        "Wo": np.ascontiguousarray(np.asarray(Wo, dtype=np.float32).astype(bf16)),
        "bq": np.ascontiguousarray(bq, dtype=np.float32),
        "bk": np.ascontiguousarray(bk, dtype=np.float32),
        "bv": np.ascontiguousarray(bv, dtype=np.float32),
        "bo": np.ascontiguousarray(bo, dtype=np.float32),
    }
    in_maps = []
    for c in range(NCORES):
        b, h = divmod(c, 2)
        xt = x[b].T  # [D, S]
        # roll so this core's query rows land at columns 0..SQ-1
        xt = np.ascontiguousarray(np.roll(xt, -h * SQ, axis=1).astype(bf16))
        in_maps.append({"xT": xt, **shared})
    return in_maps


def run(x, Wq, bq, Wk, bk, Wv, bv, Wo, bo, trace=False):
    nc = _get_nc()
    in_maps = make_in_maps(x, Wq, bq, Wk, bk, Wv, bv, Wo, bo)
    res = run_bass_kernel_spmd(
        nc, in_maps, core_ids=list(range(NCORES)), trace=trace
    )
    B = 4
    out = np.empty((B, S, D), dtype=np.float32)
    for c in range(NCORES):
        b, h = divmod(c, 2)
        out[b, h * SQ : (h + 1) * SQ, :] = res.results[c]["out"]
    return out, res


def kernel(**inputs):
    out, _ = run(**inputs)
    return out


# revision 29
# speedup vs baseline: 1.0645x; 1.0645x over previous
"""Multi-head self-attention (B=4, S=2048, D=1024, H=16) on 8 TRN2 cores.

Sharding: core c handles batch b=c//2, query rows [h*1024, (h+1)*1024) with
h=c%2, for ALL 16 heads. K/V are computed per-core over the full sequence
(duplicated across the 2 cores of a batch), so there are no collectives and
the host-side unshard is a pure concatenation.

Host prep per core: xT = x[b].T rolled so the core's queries occupy columns
0..1023 (attention is invariant to key/value ordering, so K/V over the
rolled sequence give identical outputs).

Device layout choices (everything contracts on the partition dim):
  - Q^T, K^T computed as [dq, s] / [dk, s] via lhsT=W (natural), rhs=xT.
  - scores^T [sk, sq] via lhsT=K^T tile (K=64 contraction; two heads of a
    pair live at partitions 0:64 / 64:128 and pack the PE array).
  - exp on ScalarE with fused 1/8 scale, bf16 out.
  - attn^T via lhsT=[V|1] (65 cols): row 64 accumulates the softmax
    denominators for free.
  - normalize with DVE reciprocal + GpSimd partition_broadcast + DVE mul.
  - out[sq, do] via lhsT=attn^T tile (natural output layout, contiguous DMA).
"""

import numpy as np
from contextlib import ExitStack

import concourse.bass as bass
import concourse.mybir as mybir
import concourse.tile as tile
from concourse.bass_utils import run_bass_kernel_spmd
from concourse.vector_clock import ScopedClock

F32 = mybir.dt.float32
BF16 = mybir.dt.bfloat16

D = 1024
S = 2048
SQ = 1024  # local query rows per core
H = 16
HD = 64
NPAIR = 8  # head pairs; pair p = heads (2p, 2p+1) = Q/K rows 128p..128p+127
NCORES = 8

# ---------------------------------------------------------------------------
# Workaround: this walrus build rejects >1 sem-wait per instruction ("Too
# many sync wait commands"). After the kernel is fully built, hoist excess
# waits from every instruction onto single-wait NOPs inserted just before it
# in the same engine stream (per-engine program order is preserved, so
# blocking on the NOPs first is equivalent).
# ---------------------------------------------------------------------------


def _split_all_multiwaits(nc):
    n = 0
    for fn in nc.m.functions:
        for bb in fn.blocks:
            lst = bb.instructions
            i = 0
            while i < len(lst):
                inst = lst[i]
                si = inst.sync_info
                if si is not None and si.on_wait is not None and len(si.on_wait) > 1:
                    waits = list(si.on_wait)
                    keep = waits[-1:]
                    del si.on_wait[:]
                    si.on_wait.extend(keep)
                    nops = []
                    for w in waits[:-1]:
                        nop = mybir.InstNoOp(name=f"WSPL-{n}", ins=[], outs=[])
                        n += 1
                        nop.engine = inst.engine
                        nop.sync_info = mybir.SyncInfo(on_wait=[w], on_update=[])
                        nops.append(nop)
                    lst[i:i] = nops
                    i += len(nops)
                i += 1
    return n


# ---------------------------------------------------------------------------
# Kernel build
# ---------------------------------------------------------------------------


def _bcast_ap(dram_handle, nparts, offset_elems, n):
    """DRAM AP replicating a [n] vector across nparts partitions."""
    return bass.AP(
        tensor=dram_handle,
        offset=offset_elems,
        ap=[[0, nparts], [1, n]],
    )


def build_kernel():
    nc = bass.Bass()

    xT = nc.declare_dram_parameter("xT", [D, S], BF16, isOutput=False)
    Wq = nc.declare_dram_parameter("Wq", [D, D], BF16, isOutput=False)
    Wk = nc.declare_dram_parameter("Wk", [D, D], BF16, isOutput=False)
    Wv = nc.declare_dram_parameter("Wv", [D, D], BF16, isOutput=False)
    Wo = nc.declare_dram_parameter("Wo", [D, D], BF16, isOutput=False)
    bq = nc.declare_dram_parameter("bq", [D], F32, isOutput=False)
    bk = nc.declare_dram_parameter("bk", [D], F32, isOutput=False)
    bv = nc.declare_dram_parameter("bv", [D], F32, isOutput=False)
    bo = nc.declare_dram_parameter("bo", [D], F32, isOutput=False)
    out = nc.declare_dram_parameter("out", [SQ, D], F32, isOutput=True)

    Exp = mybir.ActivationFunctionType.Exp

    with tile.TileContext(nc) as tc:
        with ExitStack() as ctx:
            const = ctx.enter_context(tc.tile_pool(name="const", bufs=1))
            xpool = ctx.enter_context(tc.tile_pool(name="xres", bufs=1))
            wqk = ctx.enter_context(tc.tile_pool(name="wqk", bufs=1))
            wv_pool = ctx.enter_context(tc.tile_pool(name="wv", bufs=2))
            qk_pool = ctx.enter_context(tc.tile_pool(name="qk", bufs=2))
            vg_pool = ctx.enter_context(tc.tile_pool(name="vg", bufs=2))
            exp_pool = ctx.enter_context(tc.tile_pool(name="expp", bufs=2))
            small = ctx.enter_context(tc.tile_pool(name="small", bufs=1))
            out_pool = ctx.enter_context(tc.tile_pool(name="outp", bufs=2))
            wo_pool = ctx.enter_context(tc.tile_pool(name="wo", bufs=3))

            spsum = ctx.enter_context(tc.tile_pool(name="sp", bufs=2, space="PSUM"))
            apsum = ctx.enter_context(tc.tile_pool(name="ap", bufs=1, space="PSUM"))
            qpsum = ctx.enter_context(tc.tile_pool(name="qp", bufs=2, space="PSUM"))
            drpool = ctx.enter_context(tc.tile_pool(name="dr", bufs=4, space="DRAM"))

            # Residents: x^T [d, s] as 8 partition-tiles; attn^T accumulator.
            xT_sb = xpool.tile([128, 8, S], BF16)
            xT_r = xT.rearrange("(k p) s -> p k s", p=128)
            for k in range(8):
                nc.sync.dma_start(out=xT_sb[:, k, :], in_=xT_r[:, k, :])

            attnT = xpool.tile([128, NPAIR, SQ], BF16)

            # Biases.
            bq_sb = const.tile([128, 8], F32)
            nc.gpsimd.dma_start(out=bq_sb[:], in_=bq.rearrange("(k p) -> p k", p=128))
            bk_sb = const.tile([128, 8], F32)
            nc.gpsimd.dma_start(out=bk_sb[:], in_=bk.rearrange("(k p) -> p k", p=128))
            bv_bc = const.tile([128, D], F32)
            nc.gpsimd.dma_start(out=bv_bc[:], in_=_bcast_ap(bv, 128, 0, D))
            bo_bc = const.tile([128, D], F32)
            nc.gpsimd.dma_start(out=bo_bc[:], in_=_bcast_ap(bo, 128, 0, D))


            def emit_vgroup(g):
                wv_g = wv_pool.tile([128, 8, 512], BF16, tag="wvg")
                nc.sync.dma_start(
                    out=wv_g[:],
                    in_=Wv[:, 512 * g : 512 * (g + 1)].rearrange(
                        "(k p) c -> p k c", p=128
                    ),
                )
                vg = vg_pool.tile([128, 16, 8, 65], BF16, tag="vg")
                # ones column (index 64) via ACT: Copy(x*0 + 1) == 1.0
                nc.scalar.activation(
                    vg[:, :, :, 64:65],
                    bv_bc[:, 0:128].rearrange("p (a b c) -> p a b c", a=16, b=8),
                    mybir.ActivationFunctionType.Copy,
                    bias=1.0,
                    scale=0.0,
                )
                for skt in range(16):
                    ps = qpsum.tile([128, 512], F32, tag="qp")
                    for k in range(8):
                        nc.tensor.matmul(
                            ps[:],
                            xT_sb[:, k, 128 * skt : 128 * (skt + 1)],
                            wv_g[:, k, :],
                            start=(k == 0),
                            stop=(k == 7),
                        )
                    nc.vector.tensor_add(
                        vg[:, skt, :, 0:64],
                        ps[:].rearrange("p (h d) -> p h d", h=8),
                        bv_bc[:, 512 * g : 512 * (g + 1)].rearrange(
                            "p (h d) -> p h d", h=8
                        ),
                    )
                return vg

            vg = emit_vgroup(0)
            vg_next = None
            for p in range(NPAIR):
                g, pl = divmod(p, 4)  # V group, pair-local index
                if p == 2:
                    vg_next = emit_vgroup(1)
                if p == 4:
                    vg = vg_next

                # --- Q^T / K^T for this pair ---
                wq_p = wqk.tile([128, 8, 128], BF16, tag="wq")
                nc.sync.dma_start(
                    out=wq_p[:],
                    in_=Wq[:, 128 * p : 128 * (p + 1)].rearrange(
                        "(k p) c -> p k c", p=128
                    ),
                )
                wk_p = wqk.tile([128, 8, 128], BF16, tag="wk")
                nc.sync.dma_start(
                    out=wk_p[:],
                    in_=Wk[:, 128 * p : 128 * (p + 1)].rearrange(
                        "(k p) c -> p k c", p=128
                    ),
                )

                qt = qk_pool.tile([128, SQ], BF16, tag="qt")
                for c in range(SQ // 512):
                    ps = qpsum.tile([128, 512], F32, tag="qp")
                    for k in range(8):
                        nc.tensor.matmul(
                            ps[:],
                            wq_p[:, k, :],
                            xT_sb[:, k, 512 * c : 512 * (c + 1)],
                            start=(k == 0),
                            stop=(k == 7),
                        )
                    nc.vector.tensor_scalar_add(
                        qt[:, 512 * c : 512 * (c + 1)], ps[:], bq_sb[:, p : p + 1]
                    )

                kt = qk_pool.tile([128, S], BF16, tag="kt")
                for c in range(S // 512):
                    ps = qpsum.tile([128, 512], F32, tag="qp")
                    for k in range(8):
                        nc.tensor.matmul(
                            ps[:],
                            wk_p[:, k, :],
                            xT_sb[:, k, 512 * c : 512 * (c + 1)],
                            start=(k == 0),
                            stop=(k == 7),
                        )
                    nc.vector.tensor_scalar_add(
                        kt[:, 512 * c : 512 * (c + 1)], ps[:], bk_sb[:, p : p + 1]
                    )

                # --- attention for the two heads of this pair ---
                sums97 = small.tile([97, 512], F32, tag="sums")
                for cq in range(SQ // 512):
                    attnA = apsum.tile([65, 512], F32, tag="aA")
                    attnB = apsum.tile([65, 512], F32, tag="aB")
                    for skt in range(16):
                        sc = spsum.tile([128, 1024], F32, tag="sc")
                        nc.tensor.matmul(
                            sc[:, 0:512],
                            kt[0:64, 128 * skt : 128 * (skt + 1)],
                            qt[0:64, 512 * cq : 512 * (cq + 1)],
                            start=True,
                            stop=True,
                        )
                        nc.tensor.matmul(
                            sc[:, 512:1024],
                            kt[64:128, 128 * skt : 128 * (skt + 1)],
                            qt[64:128, 512 * cq : 512 * (cq + 1)],
                            start=True,
                            stop=True,
                        )
                        ex = exp_pool.tile([128, 1024], BF16, tag="ex")
                        nc.scalar.activation(ex[:], sc[:], Exp, scale=0.125)
                        nc.tensor.matmul(
                            attnA[:],
                            vg[:, skt, 2 * pl, :],
                            ex[:, 0:512],
                            start=(skt == 0),
                            stop=(skt == 15),
                        )
                        nc.tensor.matmul(
                            attnB[:],
                            vg[:, skt, 2 * pl + 1, :],
                            ex[:, 512:1024],
                            start=(skt == 0),
                            stop=(skt == 15),
                        )

                    # Stash denominators (row 64) and evict attn^T
                    # unnormalized right away so the accumulation banks free
                    # quickly; normalization happens once per pair below.
                    for half, at in ((0, attnA), (1, attnB)):
                        r = 32 * (2 * cq + half)
                        nc.vector.tensor_copy(
                            sums97[r : r + 1, :], at[64:65, :]
                        )
                        nc.vector.tensor_copy(
                            attnT[
                                64 * half : 64 * (half + 1),
                                p,
                                512 * cq : 512 * (cq + 1),
                            ],
                            at[0:64, :],
                        )

                # One batched reciprocal for the pair's 4 (cq, head) chunks,
                # partition-broadcast via a DRAM round-trip (cast to bf16),
                # then normalize attn^T in place.
                rr97 = small.tile([97, 512], F32, tag="rr")
                nc.vector.reciprocal(rr97[:], sums97[:])
                scr = drpool.tile([4, 512], F32, tag="scr")
                nc.sync.dma_start(
                    out=scr[:],
                    in_=bass.AP(
                        tensor=rr97.tensor,
                        offset=rr97.offset,
                        ap=[[32 * 512, 4], [1, 512]],
                    ),
                )
                bc4 = small.tile([128, 4, 512], BF16, tag="bc")
                nc.gpsimd.dma_start(
                    out=bc4[:],
                    in_=bass.AP(
                        tensor=scr.tensor,
                        offset=scr.offset,
                        ap=[[0, 128], [512, 4], [1, 512]],
                    ),
                )
                for cq in range(SQ // 512):
                    for half in range(2):
                        sl = attnT[
                            64 * half : 64 * (half + 1),
                            p,
                            512 * cq : 512 * (cq + 1),
                        ]
                        nc.vector.tensor_mul(
                            sl,
                            sl,
                            bc4[64 * half : 64 * (half + 1), 2 * cq + half, :],
                        )



            # --- output projection: out[sq, do] += attn^T.T @ Wo ---
            OC = 512
            for c in range(D // OC):
                wo_c = wo_pool.tile([128, 8, OC], BF16, tag="wo")
                nc.sync.dma_start(
                    out=wo_c[:],
                    in_=Wo[:, OC * c : OC * (c + 1)].rearrange(
                        "(k p) c -> p k c", p=128
                    ),
                )
                for t in range(SQ // 128):
                    ps = qpsum.tile([128, OC], F32, tag="qp")
                    for k in range(8):
                        nc.tensor.matmul(
                            ps[:],
                            attnT[:, k, 128 * t : 128 * (t + 1)],
                            wo_c[:, k, :],
                            start=(k == 0),
                            stop=(k == 7),
                        )
                    ot = out_pool.tile([128, OC], F32, tag="ot")
                    nc.vector.tensor_add(
                        ot[:], ps[:], bo_bc[:, OC * c : OC * (c + 1)]
                    )
                    nc.sync.dma_start(
                        out=out[128 * t : 128 * (t + 1), OC * c : OC * (c + 1)],
                        in_=ot[:],
                    )

    _split_all_multiwaits(nc)
    return nc


_NC_CACHE = None


def _get_nc():
    global _NC_CACHE
    if _NC_CACHE is None:
        _NC_CACHE = build_kernel()
    return _NC_CACHE


def make_in_maps(x, Wq, bq, Wk, bk, Wv, bv, Wo, bo):
    import ml_dtypes

    bf16 = ml_dtypes.bfloat16
    x = np.asarray(x, dtype=np.float32)
    shared = {
        "Wq": np.ascontiguousarray(np.asarray(Wq, dtype=np.float32).astype(bf16)),
        "Wk": np.ascontiguousarray(np.asarray(Wk, dtype=np.float32).astype(bf16)),
        "Wv": np.ascontiguousarray(np.asarray(Wv, dtype=np.float32).astype(bf16)),
        "Wo": np.ascontiguousarray(np.asarray(Wo, dtype=np.float32).astype(bf16)),
        "bq": np.ascontiguousarray(bq, dtype=np.float32),
        "bk": np.ascontiguousarray(bk, dtype=np.float32),
        "bv": np.ascontiguousarray(bv, dtype=np.float32),
        "bo": np.ascontiguousarray(bo, dtype=np.float32),
    }
    in_maps = []
    for c in range(NCORES):
        b, h = divmod(c, 2)
        xt = x[b].T  # [D, S]
        # roll so this core's query rows land at columns 0..SQ-1
        xt = np.ascontiguousarray(np.roll(xt, -h * SQ, axis=1).astype(bf16))
        in_maps.append({"xT": xt, **shared})
    return in_maps


def run(x, Wq, bq, Wk, bk, Wv, bv, Wo, bo, trace=False):
    nc = _get_nc()
    in_maps = make_in_maps(x, Wq, bq, Wk, bk, Wv, bv, Wo, bo)
    res = run_bass_kernel_spmd(
        nc, in_maps, core_ids=list(range(NCORES)), trace=trace
    )
    B = 4
    out = np.empty((B, S, D), dtype=np.float32)
    for c in range(NCORES):
        b, h = divmod(c, 2)
        out[b, h * SQ : (h + 1) * SQ, :] = res.results[c]["out"]
    return out, res


def kernel(**inputs):
    out, _ = run(**inputs)
    return out
